# revision 1
# baseline (speedup 1.0000x reference)
"""MAM dense kernel for Trainium2 (8 NeuronCores, SPMD data-parallel over M).

C[m,n] = max_k(x[m,k]*w[n,k]) + min_k(x[m,k]*w[n,k]) + bias[n]

Strategy per core (M_c = 512 rows of x):
  - Layout: n on partitions (8 tiles of 128 n's), k on the free axis.
  - For each group of J m-rows: broadcast those rows across all 128
    partitions via a stride-0 DMA from DRAM, then on the Vector engine:
      q = w * x_bcast            (tensor_tensor mult)
      max tree: log2(K) rounds of pairwise tensor_tensor max over halves
      min tree: same with min
    fp16 tiles run the tree rounds in the DVE 2x_1P perf mode.
  - Combine max+min+bias in fp32, store transposed output [N, M_c];
    the host transposes back and concatenates core results.

PRECISION:
  'a' — cast x,w to fp16; fp16 products (fastest, rel err ~2e-3)
  'b' — fp32 inputs, products rounded to fp16 (rel err ~1e-3)
  'c' — all fp32 (bit-exact vs fp32 reference, slowest)
"""

import os
import sys

sys.path.insert(0, "/opt/trn_rl_repo")

import numpy as np

M, K, N = 4096, 1024, 1024
N_CORES = 8
M_C = M // N_CORES  # 512 rows per core
NT = N // 128  # 8 n-tiles

PRECISION = "a"

_last_results = None  # BassKernelResults from the most recent run (for test.py)


def _build_nc(n_groups=None, nt=NT, j=None, k=K, precision=None):
    import concourse.bacc as bacc
    import concourse.mybir as mybir
    import concourse.tile as tile
    from contextlib import ExitStack

    precision = precision or PRECISION
    # fp32 tiles are twice the size; halve the group to fit SBUF
    if j is None:
        j = 2 if precision == "c" else 4
    if n_groups is None:
        n_groups = M_C // j

    f32 = mybir.dt.float32
    f16 = mybir.dt.float16
    mult = mybir.AluOpType.mult
    amax = mybir.AluOpType.max
    amin = mybir.AluOpType.min
    aadd = mybir.AluOpType.add

    in_dt = f16 if precision == "a" else f32  # dtype of w/x operand tiles
    q_dt = f32 if precision == "c" else f16  # dtype of products + trees
    in_sz = 2 if precision == "a" else 4

    m_c = n_groups * j
    n_total = nt * 128

    nc = bacc.Bacc("TRN2", target_bir_lowering=False, debug=False)
    x_d = nc.dram_tensor("x", [m_c, k], f32, kind="ExternalInput").ap()
    w_d = nc.dram_tensor("w", [n_total, k], f32, kind="ExternalInput").ap()
    b_d = nc.dram_tensor("b", [n_total], f32, kind="ExternalInput").ap()
    o_d = nc.dram_tensor("o", [n_total, m_c], f32, kind="ExternalOutput").ap()
    # broadcast-source copy of x in the operand dtype
    xs_d = nc.dram_tensor("xsd", [m_c, k], in_dt).ap()

    with tile.TileContext(nc) as tc, ExitStack() as ctx:
        p_const = ctx.enter_context(tc.tile_pool(name="const", bufs=1))

        # --- preamble: load w (+ cast), stage x into xs_d (broadcast source).
        # No slot reuse here: the direct2d DMA encoding supports one wait.
        w_sb = p_const.tile([128, nt, k], in_dt)
        b_sb = p_const.tile([128, nt], f32)
        out_sb = p_const.tile([128, nt, m_c], f32)
        with tc.tile_pool(name="stage", bufs=1) as p_stage:
            # x roundtrip first: the first broadcast DMA depends on it
            if in_dt is f16:
                x32 = p_stage.tile([128, j, k], f32)
                x16t = p_stage.tile([128, j, k], f16)
                nc.sync.dma_start(
                    x32[:n_groups], x_d.rearrange("(p jj) k -> p jj k", jj=j)
                )
                nc.vector.tensor_copy(x16t[:n_groups], x32[:n_groups])
                nc.sync.dma_start(
                    xs_d.rearrange("(p jj) k -> p jj k", jj=j), x16t[:n_groups]
                )

                w32 = p_stage.tile([128, nt, k], f32)
                nc.sync.dma_start(w32[:], w_d.rearrange("(t p) k -> p t k", p=128))
                nc.vector.tensor_copy(w_sb[:], w32[:])
            else:
                # straight fp32 copy of x to the broadcast scratch (chunks
                # of <=128 partition-rows; n_groups can exceed 128)
                x_v = x_d.rearrange("(p jj) k -> p jj k", jj=j)
                xs_v = xs_d.rearrange("(p jj) k -> p jj k", jj=j)
                for base in range(0, n_groups, 128):
                    c = min(128, n_groups - base)
                    xcp = p_stage.tile([128, j, k], f32, tag=f"xcp{base}")
                    nc.sync.dma_start(xcp[:c], x_v[base : base + c])
                    nc.sync.dma_start(xs_v[base : base + c], xcp[:c])
                nc.sync.dma_start(w_sb[:], w_d.rearrange("(t p) k -> p t k", p=128))

            nc.sync.dma_start(b_sb[:], b_d.rearrange("(t p) -> p t", p=128))

        p_xb = ctx.enter_context(
            tc.tile_pool(name="xb", bufs=3 if precision == "a" else 2)
        )
        p_q = ctx.enter_context(tc.tile_pool(name="q", bufs=1))
        p_a = ctx.enter_context(tc.tile_pool(name="ta", bufs=1))
        p_b = ctx.enter_context(tc.tile_pool(name="tb", bufs=1))
        p_r = ctx.enter_context(tc.tile_pool(name="r", bufs=2))

        w_b = w_sb[:].unsqueeze(2).broadcast_to([128, nt, j, k])

        for g in range(n_groups):
            # broadcast this group's j rows of x to all partitions (from DRAM)
            xb = p_xb.tile([128, j, k], in_dt)
            src = (
                xs_d[g * j : (g + 1) * j, :]
                .rearrange("j k -> (j k)")
                .unsqueeze(0)
                .broadcast_to([128, j * k])
            )
            nc.sync.dma_start(xb[:].rearrange("p j k -> p (j k)"), src)

            # products: q[p_n, t, jj, k] = w[p_n, t, k] * x[g*j+jj, k]
            q = p_q.tile([128, nt, j, k], q_dt)
            xb_b = xb[:].unsqueeze(1).broadcast_to([128, nt, j, k])
            nc.vector.tensor_tensor(q[:], w_b, xb_b, mult)

            # pairwise-halves reduction trees, tensor_reduce tail at f=16
            ta = p_a.tile([128, nt, j, k // 2], q_dt)
            tb = p_b.tile([128, nt, j, k // 4], q_dt)
            results = {}
            for op_name, op in (("mx", amax), ("mn", amin)):
                res = p_r.tile([128, nt, j], f32, tag=op_name)
                cur = q[:]
                f = k // 2
                use_a = True
                while f >= 16:
                    dst = (ta if use_a else tb)[:, :, :, 0:f]
                    nc.vector.tensor_tensor(
                        dst, cur[:, :, :, 0:f], cur[:, :, :, f : 2 * f], op
                    )
                    cur = dst
                    use_a = not use_a
                    f //= 2
                nc.vector.tensor_reduce(
                    res[:], cur[:, :, :, 0 : 2 * f], axis=mybir.AxisListType.X, op=op
                )
                results[op_name] = res

            # combine: out[n, m] = max + min (bias folded in at the end)
            nc.vector.tensor_tensor(
                out_sb[:, :, g * j : (g + 1) * j],
                results["mx"][:],
                results["mn"][:],
                aadd,
            )
            # halfway through, add bias to + store the finished half so the
            # output DMA overlaps the second half's compute
            if g + 1 == n_groups // 2:
                half = (n_groups // 2) * j
                bias_h = b_sb[:].unsqueeze(2).broadcast_to([128, nt, half])
                nc.vector.tensor_tensor(
                    out_sb[:, :, :half], out_sb[:, :, :half], bias_h, aadd
                )
                nc.sync.dma_start(
                    o_d.rearrange("(t p) m -> p t m", p=128)[:, :, :half],
                    out_sb[:, :, :half],
                )

        # --- bias + store for the second half
        half = (n_groups // 2) * j
        bias_h = b_sb[:].unsqueeze(2).broadcast_to([128, nt, m_c - half])
        nc.vector.tensor_tensor(
            out_sb[:, :, half:], out_sb[:, :, half:], bias_h, aadd
        )
        nc.sync.dma_start(
            o_d.rearrange("(t p) m -> p t m", p=128)[:, :, half:],
            out_sb[:, :, half:],
        )

    nc.compile()
    return nc


def kernel(x: np.ndarray, weight: np.ndarray, bias: np.ndarray) -> np.ndarray:
    global _last_results
    from concourse.bass_utils import run_bass_kernel_spmd

    try:  # NTFF tracing needs antenv.axon_hooks; disable if unavailable
        import antenv.axon_hooks  # noqa: F401
    except ImportError:
        os.environ["BASS_NEVER_TRACE"] = "1"

    x = np.ascontiguousarray(x, dtype=np.float32)
    weight = np.ascontiguousarray(weight, dtype=np.float32)
    bias = np.ascontiguousarray(bias, dtype=np.float32)

    nc = _build_nc()
    core_ids = list(range(N_CORES))
    in_maps = [
        {"x": x[c * M_C : (c + 1) * M_C], "w": weight, "b": bias} for c in core_ids
    ]
    res = run_bass_kernel_spmd(nc, in_maps, core_ids)
    _last_results = res

    out = np.empty((M, N), dtype=np.float32)
    for c in core_ids:
        out[c * M_C : (c + 1) * M_C, :] = res.results[c]["o"].T
    return out



# revision 7
# speedup vs baseline: 1.2810x; 1.2810x over previous
"""MAM dense kernel for Trainium2 (8 NeuronCores, SPMD data-parallel over M).

C[m,n] = max_k(x[m,k]*w[n,k]) + min_k(x[m,k]*w[n,k]) + bias[n]

Strategy per core (M_c = 512 rows of x), k-on-partitions layout:
  - Host pre-transposes x and w; the kernel loads
      wt  [k_p=128, kt=8, n=1024]  (w transposed; f32 + f16 copies)
      xt  [k_p=128, kt=8, m=512]   (x transposed, f32)
    so x[m, kt*128+k_p] is a per-partition scalar.
  - Per m-row, three engines pipeline the work:
      Act:  products q[k_p, kt, n] = wt * x_scalar via activation(Copy,
            scale=xt[:, kt, m]) for 7 of 8 kt tiles (f32 in -> f16 out)
      DVE:  one product tile via tensor_scalar (f16 4x mode), kt-fold
            rounds 8->4->2->1 (f16 TT, 2x mode)
      PE:   transposes the [128, 1024] max/min partials to n-on-partition
            fp16 PSUM tiles (transpose keeps lhsT dtype, so reads stay 2x)
      DVE:  folds the transposed [128, 8, 128] partials over k_p with two
            TT rounds + a tensor_reduce tail into [128 n_p, 8 nb] results
            collected per 128-row block
  - Per block: combine max+min+bias (f32), DMA out as o_alt[n_p, nb, m];
    the host transposes o_alt back to [m, n] (cheap numpy reshape).
"""

import os
import sys

sys.path.insert(0, "/opt/trn_rl_repo")

import numpy as np

M, K, N = 4096, 1024, 1024
N_CORES = 8
M_C = M // N_CORES  # 512 rows per core
KT = K // 128  # 8 k-tiles
NB = N // 128  # 8 n-tiles

# how many of the 8 product tiles the DVE computes (rest go to Act)
DVE_PROD_TILES = 1

_last_results = None  # BassKernelResults from the most recent run (for test.py)


def _build_nc():
    import concourse.bacc as bacc
    import concourse.mybir as mybir
    import concourse.tile as tile
    from concourse.masks import make_identity
    from contextlib import ExitStack

    f32 = mybir.dt.float32
    f16 = mybir.dt.float16
    mult = mybir.AluOpType.mult
    amax = mybir.AluOpType.max
    amin = mybir.AluOpType.min
    aadd = mybir.AluOpType.add
    CopyF = mybir.ActivationFunctionType.Copy

    nd = DVE_PROD_TILES
    n_blocks = M_C // 128

    nc = bacc.Bacc("TRN2", target_bir_lowering=False, debug=False)
    wt_d = nc.dram_tensor("wt", [K, N], f32, kind="ExternalInput").ap()
    xt_d = nc.dram_tensor("xt", [K, M_C], f32, kind="ExternalInput").ap()
    b_d = nc.dram_tensor("b", [N], f32, kind="ExternalInput").ap()
    # output in (n_p, nb, m) layout; host transposes back to [m, n]
    o_d = nc.dram_tensor("o", [128, NB, M_C], f32, kind="ExternalOutput").ap()

    with tile.TileContext(nc) as tc, ExitStack() as ctx:
        p_const = ctx.enter_context(tc.tile_pool(name="const", bufs=1))

        # --- preamble: load wt (f32 + f16 cast), xt, bias (n-layout)
        wt32 = p_const.tile([128, KT, N], f32)
        wt16 = p_const.tile([128, KT, N], f16)
        xt_sb = p_const.tile([128, KT, M_C], f32)
        bias_t = p_const.tile([128, NB], f32)
        ident = p_const.tile([128, 128], f16)
        nc.sync.dma_start(wt32[:], wt_d.rearrange("(kt p) n -> p kt n", p=128))
        nc.sync.dma_start(xt_sb[:], xt_d.rearrange("(kt p) m -> p kt m", p=128))
        nc.vector.tensor_copy(wt16[:], wt32[:])
        nc.sync.dma_start(bias_t[:], b_d.rearrange("(nb p) -> p nb", p=128))
        make_identity(nc, ident)

        p_q = ctx.enter_context(tc.tile_pool(name="q", bufs=2))
        p_t1 = ctx.enter_context(tc.tile_pool(name="t1", bufs=1))
        p_t3 = ctx.enter_context(tc.tile_pool(name="t3", bufs=2))
        p_acc = ctx.enter_context(tc.tile_pool(name="acc", bufs=2))
        p_out = ctx.enter_context(tc.tile_pool(name="out", bufs=2))
        p_ps = ctx.enter_context(tc.psum_pool(name="ps", bufs=2))

        H = KT // 2 * N  # flat half size (4096)

        for b in range(n_blocks):
            # block collectors: column mm holds row m's [n_p, nb] results
            mxc = p_acc.tile([128, NB, 128], f32, tag="mxc")
            mnc = p_acc.tile([128, NB, 128], f32, tag="mnc")
            for mm in range(128):
                m = b * 128 + mm
                q = p_q.tile([128, KT, N], f16, tag="q")
                # products: DVE does nd tiles (tensor_scalar, 4x), Act the rest
                for kt in range(nd):
                    nc.vector.tensor_scalar(
                        q[:, kt], wt16[:, kt], xt_sb[:, kt, m : m + 1], None, mult
                    )
                for kt in range(nd, KT):
                    nc.scalar.activation(
                        q[:, kt], wt32[:, kt], CopyF, scale=xt_sb[:, kt, m : m + 1]
                    )
                qf = q[:].rearrange("p kt n -> p (kt n)")
                a1 = p_t1.tile([128, H], f16, tag="a1")
                b1 = p_t1.tile([128, H], f16, tag="b1")
                a2 = p_t1.tile([128, 2 * N], f16, tag="a2")
                b2 = p_t1.tile([128, 2 * N], f16, tag="b2")
                a3 = p_t3.tile([128, N], f16, tag="a3")
                b3 = p_t3.tile([128, N], f16, tag="b3")
                # kt-fold rounds 8->4->2->1 (f16 TT, 2x)
                nc.vector.tensor_tensor(a1[:], qf[:, 0:H], qf[:, H : 2 * H], amax)
                nc.vector.tensor_tensor(b1[:], qf[:, 0:H], qf[:, H : 2 * H], amin)
                a1f, b1f = a1[:], b1[:]
                nc.vector.tensor_tensor(a2[:], a1f[:, 0 : 2 * N], a1f[:, 2 * N : 4 * N], amax)
                nc.vector.tensor_tensor(b2[:], b1f[:, 0 : 2 * N], b1f[:, 2 * N : 4 * N], amin)
                nc.vector.tensor_tensor(a3[:], a2[:, 0:N], a2[:, N : 2 * N], amax)
                nc.vector.tensor_tensor(b3[:], b2[:, 0:N], b2[:, N : 2 * N], amin)
                # PE: transpose partials to n-on-partition fp16 PSUM tiles
                pax = p_ps.tile([128, NB, 128], f16, tag="pax")
                pbx = p_ps.tile([128, NB, 128], f16, tag="pbx")
                a3v = a3[:].rearrange("p (nb n) -> p nb n", nb=NB)
                b3v = b3[:].rearrange("p (nb n) -> p nb n", nb=NB)
                for nb in range(NB):
                    nc.tensor.transpose(pax[:, nb], a3v[:, nb], ident[:])
                    nc.tensor.transpose(pbx[:, nb], b3v[:, nb], ident[:])
                # DVE: fold k_p 128->1 straight from PSUM (single-input rule)
                nc.vector.tensor_reduce(
                    mxc[:, :, mm : mm + 1], pax[:], axis=mybir.AxisListType.X, op=amax
                )
                nc.vector.tensor_reduce(
                    mnc[:, :, mm : mm + 1], pbx[:], axis=mybir.AxisListType.X, op=amin
                )
            # block epilogue on DVE: combine + bias, DMA out
            out_sb = p_out.tile([128, NB, 128], f32, tag="out")
            nc.vector.tensor_tensor(out_sb[:], mxc[:], mnc[:], aadd)
            nc.vector.tensor_tensor(
                out_sb[:],
                out_sb[:],
                bias_t[:].unsqueeze(2).broadcast_to([128, NB, 128]),
                aadd,
            )
            nc.sync.dma_start(o_d[:, :, b * 128 : (b + 1) * 128], out_sb[:])

    nc.compile()
    return nc


def kernel(x: np.ndarray, weight: np.ndarray, bias: np.ndarray) -> np.ndarray:
    global _last_results
    from concourse.bass_utils import run_bass_kernel_spmd

    try:  # NTFF tracing needs antenv.axon_hooks; disable if unavailable
        import antenv.axon_hooks  # noqa: F401
    except ImportError:
        os.environ["BASS_NEVER_TRACE"] = "1"

    x = np.ascontiguousarray(x, dtype=np.float32)
    weight = np.ascontiguousarray(weight, dtype=np.float32)
    bias = np.ascontiguousarray(bias, dtype=np.float32)

    wt = np.ascontiguousarray(weight.T)  # [K, N]

    nc = _build_nc()
    core_ids = list(range(N_CORES))
    in_maps = [
        {
            "wt": wt,
            "xt": np.ascontiguousarray(x[c * M_C : (c + 1) * M_C].T),  # [K, M_C]
            "b": bias,
        }
        for c in core_ids
    ]
    res = run_bass_kernel_spmd(nc, in_maps, core_ids)
    _last_results = res

    out = np.empty((M, N), dtype=np.float32)
    for c in core_ids:
        # o_alt[n_p, nb, m] -> out[m, nb*128 + n_p]
        o_alt = res.results[c]["o"]
        out[c * M_C : (c + 1) * M_C, :] = o_alt.transpose(2, 1, 0).reshape(M_C, N)
    return out


# revision 9
# speedup vs baseline: 1.3296x; 1.0379x over previous
"""MAM dense kernel for Trainium2 (8 NeuronCores, SPMD data-parallel over M).

C[m,n] = max_k(x[m,k]*w[n,k]) + min_k(x[m,k]*w[n,k]) + bias[n]

Strategy per core (M_c = 512 rows of x), k-on-partitions layout:
  - Host pre-transposes x and w; the kernel loads
      wt  [k_p=128, kt=8, n=1024]  (w transposed; f32 + f16 copies)
      xt  [k_p=128, kt=8, m=512]   (x transposed, f32)
    so x[m, kt*128+k_p] is a per-partition scalar.
  - Per m-row, three engines pipeline the work:
      Act:  products q[k_p, kt, n] = wt * x_scalar via activation(Copy,
            scale=xt[:, kt, m]) for 7 of 8 kt tiles (f32 in -> f16 out)
      DVE:  one product tile via tensor_scalar (f16 4x mode), kt-fold
            rounds 8->4->2->1 (f16 TT, 2x mode)
      PE:   transposes the [128, 1024] max/min partials to n-on-partition
            fp16 PSUM tiles (transpose keeps lhsT dtype, so reads stay 2x)
      DVE:  folds the transposed [128, 8, 128] partials over k_p with two
            TT rounds + a tensor_reduce tail into [128 n_p, 8 nb] results
            collected per 128-row block
  - Per block: combine max+min+bias (f32), DMA out as o_alt[n_p, nb, m];
    the host transposes o_alt back to [m, n] (cheap numpy reshape).
"""

import os
import sys

sys.path.insert(0, "/opt/trn_rl_repo")

import numpy as np

M, K, N = 4096, 1024, 1024
N_CORES = 8
M_C = M // N_CORES  # 512 rows per core
KT = K // 128  # 8 k-tiles
NB = N // 128  # 8 n-tiles

# how many of the 8 product tiles the DVE computes (rest go to Act)
DVE_PROD_TILES = 0

_last_results = None  # BassKernelResults from the most recent run (for test.py)


def _build_nc():
    import concourse.bacc as bacc
    import concourse.mybir as mybir
    import concourse.tile as tile
    from concourse.masks import make_identity
    from contextlib import ExitStack

    f32 = mybir.dt.float32
    f16 = mybir.dt.float16
    mult = mybir.AluOpType.mult
    amax = mybir.AluOpType.max
    amin = mybir.AluOpType.min
    aadd = mybir.AluOpType.add
    CopyF = mybir.ActivationFunctionType.Copy

    nd = DVE_PROD_TILES
    n_blocks = M_C // 128

    nc = bacc.Bacc("TRN2", target_bir_lowering=False, debug=False)
    wt_d = nc.dram_tensor("wt", [K, N], f32, kind="ExternalInput").ap()
    xt_d = nc.dram_tensor("xt", [K, M_C], f32, kind="ExternalInput").ap()
    b_d = nc.dram_tensor("b", [N], f32, kind="ExternalInput").ap()
    # output in (n_p, nb, m) layout; host transposes back to [m, n]
    o_d = nc.dram_tensor("o", [128, NB, M_C], f32, kind="ExternalOutput").ap()

    with tile.TileContext(nc) as tc, ExitStack() as ctx:
        p_const = ctx.enter_context(tc.tile_pool(name="const", bufs=1))

        # --- preamble: load wt (f32 + f16 cast), xt, bias (n-layout)
        wt32 = p_const.tile([128, KT, N], f32)
        wt16 = p_const.tile([128, KT, N], f16)
        xt_sb = p_const.tile([128, KT, M_C], f32)
        bias_t = p_const.tile([128, NB], f32)
        ident = p_const.tile([128, 128], f16)
        nc.sync.dma_start(wt32[:], wt_d.rearrange("(kt p) n -> p kt n", p=128))
        nc.sync.dma_start(xt_sb[:], xt_d.rearrange("(kt p) m -> p kt m", p=128))
        nc.vector.tensor_copy(wt16[:], wt32[:])
        nc.sync.dma_start(bias_t[:], b_d.rearrange("(nb p) -> p nb", p=128))
        make_identity(nc, ident)

        p_q = ctx.enter_context(tc.tile_pool(name="q", bufs=2))
        p_t1 = ctx.enter_context(tc.tile_pool(name="t1", bufs=1))
        p_t3 = ctx.enter_context(tc.tile_pool(name="t3", bufs=2))
        p_acc = ctx.enter_context(tc.tile_pool(name="acc", bufs=2))
        p_out = ctx.enter_context(tc.tile_pool(name="out", bufs=2))
        p_ps = ctx.enter_context(tc.psum_pool(name="ps", bufs=2))

        H = KT // 2 * N  # flat half size (4096)

        for b in range(n_blocks):
            # block collectors: column mm holds row m's [n_p, nb] results
            mxc = p_acc.tile([128, NB, 128], f32, tag="mxc")
            mnc = p_acc.tile([128, NB, 128], f32, tag="mnc")
            for mm in range(128):
                m = b * 128 + mm
                q = p_q.tile([128, KT, N], f16, tag="q")
                # products: DVE does nd tiles (tensor_scalar, 4x), Act the rest
                for kt in range(nd):
                    nc.vector.tensor_scalar(
                        q[:, kt], wt16[:, kt], xt_sb[:, kt, m : m + 1], None, mult
                    )
                for kt in range(nd, KT):
                    nc.scalar.activation(
                        q[:, kt], wt32[:, kt], CopyF, scale=xt_sb[:, kt, m : m + 1]
                    )
                qf = q[:].rearrange("p kt n -> p (kt n)")
                a1 = p_t1.tile([128, H], f16, tag="a1")
                b1 = p_t1.tile([128, H], f16, tag="b1")
                a2 = p_t1.tile([128, 2 * N], f16, tag="a2")
                b2 = p_t1.tile([128, 2 * N], f16, tag="b2")
                a3 = p_t3.tile([128, N], f16, tag="a3")
                b3 = p_t3.tile([128, N], f16, tag="b3")
                # kt-fold rounds 8->4->2->1 (f16 TT, 2x)
                nc.vector.tensor_tensor(a1[:], qf[:, 0:H], qf[:, H : 2 * H], amax)
                nc.vector.tensor_tensor(b1[:], qf[:, 0:H], qf[:, H : 2 * H], amin)
                a1f, b1f = a1[:], b1[:]
                nc.vector.tensor_tensor(a2[:], a1f[:, 0 : 2 * N], a1f[:, 2 * N : 4 * N], amax)
                nc.vector.tensor_tensor(b2[:], b1f[:, 0 : 2 * N], b1f[:, 2 * N : 4 * N], amin)
                nc.vector.tensor_tensor(a3[:], a2[:, 0:N], a2[:, N : 2 * N], amax)
                nc.vector.tensor_tensor(b3[:], b2[:, 0:N], b2[:, N : 2 * N], amin)
                # PE: transpose partials to n-on-partition fp16 PSUM tiles
                pax = p_ps.tile([128, NB, 128], f16, tag="pax")
                pbx = p_ps.tile([128, NB, 128], f16, tag="pbx")
                a3v = a3[:].rearrange("p (nb n) -> p nb n", nb=NB)
                b3v = b3[:].rearrange("p (nb n) -> p nb n", nb=NB)
                for nb in range(NB):
                    nc.tensor.transpose(pax[:, nb], a3v[:, nb], ident[:])
                    nc.tensor.transpose(pbx[:, nb], b3v[:, nb], ident[:])
                # DVE: fold k_p 128->1 straight from PSUM (single-input rule)
                nc.vector.tensor_reduce(
                    mxc[:, :, mm : mm + 1], pax[:], axis=mybir.AxisListType.X, op=amax
                )
                nc.vector.tensor_reduce(
                    mnc[:, :, mm : mm + 1], pbx[:], axis=mybir.AxisListType.X, op=amin
                )
            # block epilogue on DVE: combine + bias, DMA out
            out_sb = p_out.tile([128, NB, 128], f32, tag="out")
            nc.vector.tensor_tensor(out_sb[:], mxc[:], mnc[:], aadd)
            nc.vector.tensor_tensor(
                out_sb[:],
                out_sb[:],
                bias_t[:].unsqueeze(2).broadcast_to([128, NB, 128]),
                aadd,
            )
            nc.sync.dma_start(o_d[:, :, b * 128 : (b + 1) * 128], out_sb[:])

    nc.compile()
    return nc


def kernel(x: np.ndarray, weight: np.ndarray, bias: np.ndarray) -> np.ndarray:
    global _last_results
    from concourse.bass_utils import run_bass_kernel_spmd

    try:  # NTFF tracing needs antenv.axon_hooks; disable if unavailable
        import antenv.axon_hooks  # noqa: F401
    except ImportError:
        os.environ["BASS_NEVER_TRACE"] = "1"

    x = np.ascontiguousarray(x, dtype=np.float32)
    weight = np.ascontiguousarray(weight, dtype=np.float32)
    bias = np.ascontiguousarray(bias, dtype=np.float32)

    wt = np.ascontiguousarray(weight.T)  # [K, N]

    nc = _build_nc()
    core_ids = list(range(N_CORES))
    in_maps = [
        {
            "wt": wt,
            "xt": np.ascontiguousarray(x[c * M_C : (c + 1) * M_C].T),  # [K, M_C]
            "b": bias,
        }
        for c in core_ids
    ]
    res = run_bass_kernel_spmd(nc, in_maps, core_ids)
    _last_results = res

    out = np.empty((M, N), dtype=np.float32)
    for c in core_ids:
        # o_alt[n_p, nb, m] -> out[m, nb*128 + n_p]
        o_alt = res.results[c]["o"]
        out[c * M_C : (c + 1) * M_C, :] = o_alt.transpose(2, 1, 0).reshape(M_C, N)
    return out


# revision 10
# speedup vs baseline: 1.3619x; 1.0243x over previous
"""MAM dense kernel for Trainium2 (8 NeuronCores, SPMD data-parallel over M).

C[m,n] = max_k(x[m,k]*w[n,k]) + min_k(x[m,k]*w[n,k]) + bias[n]

Strategy per core (M_c = 512 rows of x), k-on-partitions layout:
  - Host pre-transposes x and w; the kernel loads
      wt  [k_p=128, kt=8, n=1024]  (w transposed, f32)
      xt  [k_p=128, kt=8, m=512]   (x transposed, f32)
    so x[m, kt*128+k_p] is a per-partition scalar.
  - Rows are processed in pairs (J=2) to halve DVE instruction overhead:
      Act:  products q2[k_p, j, kt, n] = wt * x_scalar via activation(Copy,
            scale=xt[:, kt, m+j]) -- 16 instructions per pair (f32 -> f16)
      DVE:  kt-fold tree rounds 8->4->2->1 on both rows at once (f16 TT,
            2x mode, 3 instructions per op per pair)
      PE:   transposes the [128, 2, 1024] max/min partials to
            n-on-partition fp16 PSUM tiles (transpose keeps lhsT dtype)
      DVE:  one tensor_reduce per op folds k_p 128->1 for both rows
            straight from PSUM into per-block collectors
  - Per 128-row block: combine max+min+bias (f32), DMA out as
    o[n_p, nb, m]; the host transposes back to [m, n] (cheap numpy).
"""

import os
import sys

sys.path.insert(0, "/opt/trn_rl_repo")

import numpy as np

M, K, N = 4096, 1024, 1024
N_CORES = 8
M_C = M // N_CORES  # 512 rows per core
KT = K // 128  # 8 k-tiles
NB = N // 128  # 8 n-tiles

_last_results = None  # BassKernelResults from the most recent run (for test.py)


def _build_nc():
    import concourse.bacc as bacc
    import concourse.mybir as mybir
    import concourse.tile as tile
    from concourse.masks import make_identity
    from contextlib import ExitStack

    f32 = mybir.dt.float32
    f16 = mybir.dt.float16
    amax = mybir.AluOpType.max
    amin = mybir.AluOpType.min
    aadd = mybir.AluOpType.add
    CopyF = mybir.ActivationFunctionType.Copy

    n_blocks = M_C // 128

    nc = bacc.Bacc("TRN2", target_bir_lowering=False, debug=False)
    wt_d = nc.dram_tensor("wt", [K, N], f32, kind="ExternalInput").ap()
    xt_d = nc.dram_tensor("xt", [K, M_C], f32, kind="ExternalInput").ap()
    b_d = nc.dram_tensor("b", [N], f32, kind="ExternalInput").ap()
    # output in (n_p, nb, m) layout; host transposes back to [m, n]
    o_d = nc.dram_tensor("o", [128, NB, M_C], f32, kind="ExternalOutput").ap()

    with tile.TileContext(nc) as tc, ExitStack() as ctx:
        p_const = ctx.enter_context(tc.tile_pool(name="const", bufs=1))

        # --- preamble: load wt, xt, bias (n-layout), identity
        wt32 = p_const.tile([128, KT, N], f32)
        xt_sb = p_const.tile([128, KT, M_C], f32)
        bias_t = p_const.tile([128, NB], f32)
        ident = p_const.tile([128, 128], f16)
        nc.sync.dma_start(wt32[:], wt_d.rearrange("(kt p) n -> p kt n", p=128))
        nc.sync.dma_start(xt_sb[:], xt_d.rearrange("(kt p) m -> p kt m", p=128))
        nc.sync.dma_start(bias_t[:], b_d.rearrange("(nb p) -> p nb", p=128))
        make_identity(nc, ident)

        p_q = ctx.enter_context(tc.tile_pool(name="q", bufs=2))
        p_t1 = ctx.enter_context(tc.tile_pool(name="t1", bufs=1))
        p_t3 = ctx.enter_context(tc.tile_pool(name="t3", bufs=2))
        p_acc = ctx.enter_context(tc.tile_pool(name="acc", bufs=2))
        p_out = ctx.enter_context(tc.tile_pool(name="out", bufs=2))
        p_ps = ctx.enter_context(tc.psum_pool(name="ps", bufs=2))

        H = KT // 2 * N  # flat half size (4096)

        for b in range(n_blocks):
            # block collectors: column mm holds row m's [n_p, nb] results
            mxc = p_acc.tile([128, NB, 128], f32, tag="mxc")
            mnc = p_acc.tile([128, NB, 128], f32, tag="mnc")
            for mm in range(0, 128, 2):
                m = b * 128 + mm
                q2 = p_q.tile([128, 2, KT, N], f16, tag="q2")
                for j in range(2):
                    for kt in range(KT):
                        nc.scalar.activation(
                            q2[:, j, kt],
                            wt32[:, kt],
                            CopyF,
                            scale=xt_sb[:, kt, m + j : m + j + 1],
                        )
                q2f = q2[:].rearrange("p j kt n -> p j (kt n)")
                a1 = p_t1.tile([128, 2, H], f16, tag="a1")
                b1 = p_t1.tile([128, 2, H], f16, tag="b1")
                a2 = p_t1.tile([128, 2, 2 * N], f16, tag="a2")
                b2 = p_t1.tile([128, 2, 2 * N], f16, tag="b2")
                a3 = p_t3.tile([128, 2, N], f16, tag="a3")
                b3 = p_t3.tile([128, 2, N], f16, tag="b3")
                # kt-fold rounds 8->4->2->1, both rows per instruction
                nc.vector.tensor_tensor(
                    a1[:], q2f[:, :, 0:H], q2f[:, :, H : 2 * H], amax
                )
                nc.vector.tensor_tensor(
                    b1[:], q2f[:, :, 0:H], q2f[:, :, H : 2 * H], amin
                )
                nc.vector.tensor_tensor(
                    a2[:], a1[:, :, 0 : 2 * N], a1[:, :, 2 * N : 4 * N], amax
                )
                nc.vector.tensor_tensor(
                    b2[:], b1[:, :, 0 : 2 * N], b1[:, :, 2 * N : 4 * N], amin
                )
                nc.vector.tensor_tensor(a3[:], a2[:, :, 0:N], a2[:, :, N : 2 * N], amax)
                nc.vector.tensor_tensor(b3[:], b2[:, :, 0:N], b2[:, :, N : 2 * N], amin)
                # PE: transpose partials to n-on-partition fp16 PSUM tiles
                pax = p_ps.tile([128, 2, NB, 128], f16, tag="pax")
                pbx = p_ps.tile([128, 2, NB, 128], f16, tag="pbx")
                a3v = a3[:].rearrange("p j (nb n) -> p j nb n", nb=NB)
                b3v = b3[:].rearrange("p j (nb n) -> p j nb n", nb=NB)
                for j in range(2):
                    for nb in range(NB):
                        nc.tensor.transpose(pax[:, j, nb], a3v[:, j, nb], ident[:])
                        nc.tensor.transpose(pbx[:, j, nb], b3v[:, j, nb], ident[:])
                # DVE: fold k_p 128->1 for both rows straight from PSUM
                nc.vector.tensor_reduce(
                    mxc[:, :, mm : mm + 2].rearrange("p nb j -> p j nb"),
                    pax[:],
                    axis=mybir.AxisListType.X,
                    op=amax,
                )
                nc.vector.tensor_reduce(
                    mnc[:, :, mm : mm + 2].rearrange("p nb j -> p j nb"),
                    pbx[:],
                    axis=mybir.AxisListType.X,
                    op=amin,
                )
            # block epilogue on DVE: combine + bias, DMA out
            out_sb = p_out.tile([128, NB, 128], f32, tag="out")
            nc.vector.tensor_tensor(out_sb[:], mxc[:], mnc[:], aadd)
            nc.vector.tensor_tensor(
                out_sb[:],
                out_sb[:],
                bias_t[:].unsqueeze(2).broadcast_to([128, NB, 128]),
                aadd,
            )
            nc.sync.dma_start(o_d[:, :, b * 128 : (b + 1) * 128], out_sb[:])

    nc.compile()
    return nc


def kernel(x: np.ndarray, weight: np.ndarray, bias: np.ndarray) -> np.ndarray:
    global _last_results
    from concourse.bass_utils import run_bass_kernel_spmd

    try:  # NTFF tracing needs antenv.axon_hooks; disable if unavailable
        import antenv.axon_hooks  # noqa: F401
    except ImportError:
        os.environ["BASS_NEVER_TRACE"] = "1"

    x = np.ascontiguousarray(x, dtype=np.float32)
    weight = np.ascontiguousarray(weight, dtype=np.float32)
    bias = np.ascontiguousarray(bias, dtype=np.float32)

    wt = np.ascontiguousarray(weight.T)  # [K, N]

    nc = _build_nc()
    core_ids = list(range(N_CORES))
    in_maps = [
        {
            "wt": wt,
            "xt": np.ascontiguousarray(x[c * M_C : (c + 1) * M_C].T),  # [K, M_C]
            "b": bias,
        }
        for c in core_ids
    ]
    res = run_bass_kernel_spmd(nc, in_maps, core_ids)
    _last_results = res

    out = np.empty((M, N), dtype=np.float32)
    for c in core_ids:
        # o[n_p, nb, m] -> out[m, nb*128 + n_p]
        o_alt = res.results[c]["o"]
        out[c * M_C : (c + 1) * M_C, :] = o_alt.transpose(2, 1, 0).reshape(M_C, N)
    return out


# revision 12
# speedup vs baseline: 1.3845x; 1.0166x over previous
"""MAM dense kernel for Trainium2 (8 NeuronCores, SPMD data-parallel over M).

C[m,n] = max_k(x[m,k]*w[n,k]) + min_k(x[m,k]*w[n,k]) + bias[n]

Strategy per core (M_c = 512 rows of x), k-on-partitions layout:
  - Host pre-transposes x and w; the kernel loads
      wt  [k_p=128, kt=8, n=1024]  (w transposed, f32)
      xt  [k_p=128, kt=8, m=512]   (x transposed, f32)
    so x[m, kt*128+k_p] is a per-partition scalar.
  - Rows are processed in pairs (J=2) to halve DVE instruction overhead:
      Act:  products q2[k_p, j, kt, n] = wt * x_scalar via activation(Copy,
            scale=xt[:, kt, m+j]) -- 16 instructions per pair (f32 -> f16)
      DVE:  kt-fold tree rounds 8->4->2->1 on both rows at once (f16 TT,
            2x mode, 3 instructions per op per pair)
      DMA:  XBAR transpose lands the [128, 2048] max/min partials
            n-on-partition in SBUF fp16 (out[p,i,:] = in[:,i*128+p])
      DVE:  folds k_p 128->1 per op with 2x TT rounds + a reduce tail
            into per-block collectors
  - Per 128-row block: combine max+min+bias (f32), DMA out as
    o[n_p, nb, m]; the host transposes back to [m, n] (cheap numpy).
"""

import os
import sys

sys.path.insert(0, "/opt/trn_rl_repo")

import numpy as np

M, K, N = 4096, 1024, 1024
N_CORES = 8
M_C = M // N_CORES  # 512 rows per core
KT = K // 128  # 8 k-tiles
NB = N // 128  # 8 n-tiles

_last_results = None  # BassKernelResults from the most recent run (for test.py)


def _build_nc():
    import concourse.bacc as bacc
    import concourse.mybir as mybir
    import concourse.tile as tile
    from contextlib import ExitStack

    f32 = mybir.dt.float32
    f16 = mybir.dt.float16
    amax = mybir.AluOpType.max
    amin = mybir.AluOpType.min
    aadd = mybir.AluOpType.add
    CopyF = mybir.ActivationFunctionType.Copy

    n_blocks = M_C // 128

    nc = bacc.Bacc("TRN2", target_bir_lowering=False, debug=False)
    wt_d = nc.dram_tensor("wt", [K, N], f32, kind="ExternalInput").ap()
    xt_d = nc.dram_tensor("xt", [K, M_C], f32, kind="ExternalInput").ap()
    b_d = nc.dram_tensor("b", [N], f32, kind="ExternalInput").ap()
    # output in (n_p, nb, m) layout; host transposes back to [m, n]
    o_d = nc.dram_tensor("o", [128, NB, M_C], f32, kind="ExternalOutput").ap()

    with tile.TileContext(nc) as tc, ExitStack() as ctx:
        p_const = ctx.enter_context(tc.tile_pool(name="const", bufs=1))

        # --- preamble: load wt, xt, bias (n-layout)
        wt32 = p_const.tile([128, KT, N], f32)
        xt_sb = p_const.tile([128, KT, M_C], f32)
        bias_t = p_const.tile([128, NB], f32)
        nc.sync.dma_start(wt32[:], wt_d.rearrange("(kt p) n -> p kt n", p=128))
        nc.sync.dma_start(xt_sb[:], xt_d.rearrange("(kt p) m -> p kt m", p=128))
        nc.sync.dma_start(bias_t[:], b_d.rearrange("(nb p) -> p nb", p=128))

        p_q = ctx.enter_context(tc.tile_pool(name="q", bufs=2))
        p_t1 = ctx.enter_context(tc.tile_pool(name="t1", bufs=1))
        p_t3 = ctx.enter_context(tc.tile_pool(name="t3", bufs=2))
        p_acc = ctx.enter_context(tc.tile_pool(name="acc", bufs=2))
        p_out = ctx.enter_context(tc.tile_pool(name="out", bufs=2))
        p_tt = ctx.enter_context(tc.tile_pool(name="tt", bufs=2))
        p_pf = ctx.enter_context(tc.tile_pool(name="pf", bufs=1))

        H = KT // 2 * N  # flat half size (4096)

        for b in range(n_blocks):
            # block collectors: column mm holds row m's [n_p, nb] results
            mxc = p_acc.tile([128, NB, 128], f32, tag="mxc")
            mnc = p_acc.tile([128, NB, 128], f32, tag="mnc")
            for mm in range(0, 128, 2):
                m = b * 128 + mm
                q2 = p_q.tile([128, 2, KT, N], f16, tag="q2")
                for j in range(2):
                    for kt in range(KT):
                        nc.scalar.activation(
                            q2[:, j, kt],
                            wt32[:, kt],
                            CopyF,
                            scale=xt_sb[:, kt, m + j : m + j + 1],
                        )
                q2f = q2[:].rearrange("p j kt n -> p j (kt n)")
                a1 = p_t1.tile([128, 2, H], f16, tag="a1")
                b1 = p_t1.tile([128, 2, H], f16, tag="b1")
                a3 = p_t3.tile([128, 2, N], f16, tag="a3")
                b3 = p_t3.tile([128, 2, N], f16, tag="b3")
                # kt-fold rounds 8->4->2->1, both rows per instruction.
                # Round-2 outputs reuse q2's space (q2 is dead after round 1).
                a2v = q2f[:, :, 0 : 2 * N]
                b2v = q2f[:, :, 2 * N : 4 * N]
                nc.vector.tensor_tensor(
                    a1[:], q2f[:, :, 0:H], q2f[:, :, H : 2 * H], amax
                )
                nc.vector.tensor_tensor(
                    b1[:], q2f[:, :, 0:H], q2f[:, :, H : 2 * H], amin
                )
                nc.vector.tensor_tensor(
                    a2v, a1[:, :, 0 : 2 * N], a1[:, :, 2 * N : 4 * N], amax
                )
                nc.vector.tensor_tensor(
                    b2v, b1[:, :, 0 : 2 * N], b1[:, :, 2 * N : 4 * N], amin
                )
                nc.vector.tensor_tensor(a3[:], a2v[:, :, 0:N], a2v[:, :, N : 2 * N], amax)
                nc.vector.tensor_tensor(b3[:], b2v[:, :, 0:N], b2v[:, :, N : 2 * N], amin)
                # XBAR DMA transpose: partials land n-on-partition in SBUF
                # f16 (out[p, i, :] = in[:, i*128+p]; i = j*NB + nb)
                ta = p_tt.tile([128, 2 * NB, 128], f16, tag="ta")
                tb = p_tt.tile([128, 2 * NB, 128], f16, tag="tb")
                nc.sync.dma_start_transpose(ta[:], a3[:].rearrange("p j n -> p (j n)"))
                nc.sync.dma_start_transpose(tb[:], b3[:].rearrange("p j n -> p (j n)"))
                # DVE: fold k_p 128->1 with 2x TT rounds + small reduce tail
                f4 = p_pf.tile([128, 2 * NB, 64], f16, tag="f4")
                g4 = p_pf.tile([128, 2 * NB, 64], f16, tag="g4")
                nc.vector.tensor_tensor(f4[:], ta[:, :, 0:64], ta[:, :, 64:128], amax)
                nc.vector.tensor_tensor(
                    f4[:, :, 0:32], f4[:, :, 0:32], f4[:, :, 32:64], amax
                )
                nc.vector.tensor_reduce(
                    mxc[:, :, mm : mm + 2].rearrange("p nb j -> p j nb"),
                    f4[:].rearrange("p (j nb) f -> p j nb f", j=2)[:, :, :, 0:32],
                    axis=mybir.AxisListType.X,
                    op=amax,
                )
                nc.vector.tensor_tensor(g4[:], tb[:, :, 0:64], tb[:, :, 64:128], amin)
                nc.vector.tensor_tensor(
                    g4[:, :, 0:32], g4[:, :, 0:32], g4[:, :, 32:64], amin
                )
                nc.vector.tensor_reduce(
                    mnc[:, :, mm : mm + 2].rearrange("p nb j -> p j nb"),
                    g4[:].rearrange("p (j nb) f -> p j nb f", j=2)[:, :, :, 0:32],
                    axis=mybir.AxisListType.X,
                    op=amin,
                )
            # block epilogue on DVE: combine + bias, DMA out
            out_sb = p_out.tile([128, NB, 128], f32, tag="out")
            nc.vector.tensor_tensor(out_sb[:], mxc[:], mnc[:], aadd)
            nc.vector.tensor_tensor(
                out_sb[:],
                out_sb[:],
                bias_t[:].unsqueeze(2).broadcast_to([128, NB, 128]),
                aadd,
            )
            nc.sync.dma_start(o_d[:, :, b * 128 : (b + 1) * 128], out_sb[:])

    nc.compile()
    return nc


def kernel(x: np.ndarray, weight: np.ndarray, bias: np.ndarray) -> np.ndarray:
    global _last_results
    from concourse.bass_utils import run_bass_kernel_spmd

    try:  # NTFF tracing needs antenv.axon_hooks; disable if unavailable
        import antenv.axon_hooks  # noqa: F401
    except ImportError:
        os.environ["BASS_NEVER_TRACE"] = "1"

    x = np.ascontiguousarray(x, dtype=np.float32)
    weight = np.ascontiguousarray(weight, dtype=np.float32)
    bias = np.ascontiguousarray(bias, dtype=np.float32)

    wt = np.ascontiguousarray(weight.T)  # [K, N]

    nc = _build_nc()
    core_ids = list(range(N_CORES))
    in_maps = [
        {
            "wt": wt,
            "xt": np.ascontiguousarray(x[c * M_C : (c + 1) * M_C].T),  # [K, M_C]
            "b": bias,
        }
        for c in core_ids
    ]
    res = run_bass_kernel_spmd(nc, in_maps, core_ids)
    _last_results = res

    out = np.empty((M, N), dtype=np.float32)
    for c in core_ids:
        # o[n_p, nb, m] -> out[m, nb*128 + n_p]
        o_alt = res.results[c]["o"]
        out[c * M_C : (c + 1) * M_C, :] = o_alt.transpose(2, 1, 0).reshape(M_C, N)
    return out
